# revision 20
# baseline (speedup 1.0000x reference)
"""Multi-head attention (B=2, T=2048, E=1024, H=16) on 8 TRN2 NeuronCores.

Sharding: core c handles batch c//4 and head group c%4 (4 heads of 64 dims
-> 256 columns of w_Q/w_K/w_V and of the output). Pure SPMD, no collectives:
every core runs the same NEFF on its own input shard.

Per-core kernel (all matmul operands bf16, PSUM/softmax math fp32):
  xT [E, T] (host pre-transposed), wq/wk/wv [E, 256]
  1. QT/KT per head-pair p: [128, T] = (w pair-slice)^T @ xT   (PE)
  2. V per s-tile: [128, 4*65] with a ones column per head     (PE + DVE copy)
  3. scores transposed per head: ST[s, t] = K Q^T, two heads packed into
     PE row groups (K=64 each) writing one [128, 1024] PSUM tile
  4. exp via ACT straight from PSUM, scale=1/8 folded into the activation
     affine, bf16 out -> PT
  5. attn: out[t,65] = PT_slice^T @ V_aug accumulated over 16 s-chunks;
     col 64 = softmax denominator (from the ones column)
  6. normalize: DVE reciprocal + per-partition tensor_scalar mul -> fp32 out
"""

import numpy as np
import ml_dtypes

B, T, E, H = 2, 2048, 1024, 16
D = 64          # head dim
HG = 4          # heads per core
GC = HG * D     # 256 output columns per core
NCORES = 8

_cached_nc = None


def _build_program(seq: int = T, reps: int = 1, skip_attn=False, skip_exp=False):
    """reps>1 emits the body multiple times in one NEFF (timing only).
    skip_attn/skip_exp build ablation variants for HW phase attribution."""
    import concourse.bacc as bacc
    import concourse.tile as tile
    from concourse import mybir

    bf16 = mybir.dt.bfloat16
    f32 = mybir.dt.float32
    i16 = mybir.dt.int16
    Exp = mybir.ActivationFunctionType.Exp
    Mult = mybir.AluOpType.mult
    Add = mybir.AluOpType.add
    # int16 Schraudolph fast-exp: bits16 = 2^7*(x*log2e*0.125 + 127 - c),
    # written as int16 and bitcast to bf16 (bf16 shares fp32's exponent
    # layout). One tensor_scalar per group, no convert-copy needed, so DVE
    # and GPSIMD form two independent exp lanes next to ACT's.
    S16_C1 = (1 << 7) * 1.4426950408889634 * 0.125
    S16_C2 = (1 << 7) * (127.0 - 0.04367744)
    EXP_ACT_GROUPS = 9   # 1024-wide groups [0,9) -> ACT
    # groups [9,16) -> DVE int16 lane (GPSIMD cannot read PSUM)

    NT = seq // 128     # s-tiles / t-tiles
    NTC = seq // 512    # 512-wide t-chunks
    KO = E // 128       # contraction chunks for projections

    nc = bacc.Bacc(
        "TRN2", target_bir_lowering=False, debug=False, num_devices=NCORES
    )

    xT_d = nc.dram_tensor("xT", [E, seq], bf16, kind="ExternalInput")
    wq_d = nc.dram_tensor("wq", [E, GC], bf16, kind="ExternalInput")
    wk_d = nc.dram_tensor("wk", [E, GC], bf16, kind="ExternalInput")
    wv_d = nc.dram_tensor("wv", [E, GC], bf16, kind="ExternalInput")
    out_d = nc.dram_tensor("out", [seq, GC], f32, kind="ExternalOutput")

    with tile.TileContext(nc) as tc:
        with (
            tc.tile_pool(name="singles", bufs=1) as singles,
            tc.tile_pool(name="pt", bufs=3) as ptp,
            tc.tile_pool(name="stage", bufs=8) as stagep,
            tc.tile_pool(name="recip", bufs=8) as recipp,
            # PSUM budget (8 banks): scores 4x[128,512] (4) + attn 2 + proj 2.
            # proj/attn double-buffered so PSUM->SBUF drains overlap the next
            # accumulation chain; scores 4 banks deep so the ACT and DVE exp
            # lanes drain different banks CONCURRENTLY instead of being
            # serialized by tile reuse.
            tc.tile_pool(name="proj_ps", bufs=2, space="PSUM") as proj_ps,
            tc.tile_pool(name="score_ps", bufs=4, space="PSUM") as score_ps,
            tc.tile_pool(name="attn_ps", bufs=2, space="PSUM") as attn_ps,
        ):
          for _rep in range(reps):
            # ---- load inputs (weights first: first matmul needs wk+xT[0]) --
            wq = singles.tile([128, KO, GC], bf16)
            wk = singles.tile([128, KO, GC], bf16)
            wv = singles.tile([128, KO, GC], bf16)
            nc.sync.dma_start(wk[:], wk_d[:].rearrange("(ko p) c -> p ko c", p=128))
            nc.sync.dma_start(wq[:], wq_d[:].rearrange("(ko p) c -> p ko c", p=128))
            # xT streamed t-chunk-major so the first projection's k-loop only
            # waits for the first 1MB, not the full 4MB.
            xT = singles.tile([128, KO, seq], bf16)
            for tcq in range(NTC):
                for k in range(KO):
                    nc.sync.dma_start(
                        xT[:, k, tcq * 512:(tcq + 1) * 512],
                        xT_d[k * 128:(k + 1) * 128, tcq * 512:(tcq + 1) * 512],
                    )
            nc.sync.dma_start(wv[:], wv_d[:].rearrange("(ko p) c -> p ko c", p=128))

            # QT/KT: [128, pair, seq]; partitions 0-63 head 2p, 64-127 head 2p+1
            QT = singles.tile([128, 2, seq], bf16)
            KT = singles.tile([128, 2, seq], bf16)
            # V with ones col per head: [128, s-tile, 4*65]
            V = singles.tile([128, NT, HG * (D + 1)], bf16)
            # only the per-head ones-columns need the memset; the D data
            # columns are fully overwritten by proj_v
            nc.vector.memset(
                V[:].rearrange("p s (h c) -> p s h c", h=HG)[:, :, :, D:], 1.0
            )

            def proj_qk(p, w_sb, dst, tcs, eng=None):
                """Project t-chunks `tcs` of QT or KT for head-pair p.
                eng picks the PSUM->SBUF copy engine (DVE default; ACT for
                KT to balance the two engines' load)."""
                for tcq in tcs:
                    ps = proj_ps.tile([128, 512], f32, tag="proj")
                    for k in range(KO):
                        nc.tensor.matmul(
                            ps[:],
                            lhsT=w_sb[:, k, p * 128:(p + 1) * 128],
                            rhs=xT[:, k, tcq * 512:(tcq + 1) * 512],
                            start=(k == 0),
                            stop=(k == KO - 1),
                        )
                    dst_ap = dst[:, p, tcq * 512:(tcq + 1) * 512]
                    if eng == "act":
                        nc.scalar.copy(out=dst_ap, in_=ps[:])
                    else:
                        nc.vector.tensor_copy(out=dst_ap, in_=ps[:])

            def proj_v(tiles):
                for st in tiles:
                    ps = proj_ps.tile([128, 512], f32, tag="proj")
                    for k in range(KO):
                        nc.tensor.matmul(
                            ps[:, :GC],
                            lhsT=xT[:, k, st * 128:(st + 1) * 128],
                            rhs=wv[:, k, :],
                            start=(k == 0),
                            stop=(k == KO - 1),
                        )
                    nc.scalar.copy(
                        out=V[:, st].rearrange("p (h c) -> p h c", h=HG)[:, :, :D],
                        in_=ps[:, :GC].rearrange("p (h c) -> p h c", h=HG),
                    )

            def scores_unit(p, tcq, hooks=None):
                """ST = K Q^T (both heads row-packed) + exp -> PT tile.

                PT layout is flat [128, NT*1024]: 512-wide bank-write j=2*st+h
                lands at elem offset j*512 (= st*1024 + h*512). Exps are
                grouped 2 banks at a time (FD=1024). hooks[st] emits filler
                work just before score s-tile st."""
                pt = ptp.tile([128, NT * 1024], bf16, tag="pt")
                for st in range(NT):
                    for f in (hooks or {}).get(st, []):
                        f()
                    for h in range(2):
                        j = 2 * st + h
                        sc = score_ps.tile([128, 512], f32, tag="score")
                        nc.tensor.matmul(
                            sc[:],
                            lhsT=KT[h * 64:(h + 1) * 64, p,
                                    st * 128:(st + 1) * 128],
                            rhs=QT[h * 64:(h + 1) * 64, p,
                                   tcq * 512:(tcq + 1) * 512],
                            start=True,
                            stop=True,
                        )
                        if skip_exp:
                            continue
                        dst = pt[:, j * 512:(j + 1) * 512]
                        # Two exp lanes, ~9:7 ACT:DVE per 16 banks; early
                        # slices on ACT (attn consumes PT in s-order).
                        if (j % 16) < EXP_ACT_GROUPS:
                            nc.scalar.activation(
                                out=dst, in_=sc[:],
                                func=Exp, scale=0.125,
                            )
                        else:
                            nc.vector.tensor_scalar(
                                dst.bitcast(i16), sc[:],
                                S16_C1, S16_C2, Mult, Add,
                            )
                return pt

            def attn_emit(p, tcq, pt):
                """attn = PT^T @ V_aug accumulated over s, then normalize.
                Returns 8 closures, one per (h, tt) accumulation chain (heads
                sequential, one open accumulation group per PSUM bank), so
                chains can be interleaved between score groups as PE filler.
                Normalize + output DMA ride on the tt=3 chains."""
                stgs, aps = [], {}

                def mk(c):
                    h, tt = divmod(c, 4)
                    hh = p * 2 + h

                    def run():
                        if c == 0:
                            stgs.extend(
                                stagep.tile([128, 128], f32, tag="stage",
                                            name=f"stg{t}")
                                for t in range(4)
                            )
                        if tt == 0:
                            aps[h] = attn_ps.tile([128, 4 * (D + 1)], f32,
                                                  tag="attn", name=f"ap{h}")
                        ap = aps[h]
                        for st in range(NT):
                            nc.tensor.matmul(
                                ap[:, tt * (D + 1):(tt + 1) * (D + 1)],
                                lhsT=pt[:, st * 1024 + h * 512 + tt * 128:
                                        st * 1024 + h * 512 + (tt + 1) * 128],
                                rhs=V[:, st, hh * (D + 1):(hh + 1) * (D + 1)],
                                start=(st == 0),
                                stop=(st == NT - 1),
                            )
                        if tt != 3:
                            return
                        # one strided reciprocal over the 4 denom columns
                        r = recipp.tile([128, 4], f32, tag="recip")
                        nc.vector.reciprocal(
                            out=r[:],
                            in_=ap[:].rearrange(
                                "p (t c) -> p t c", c=D + 1)[:, :, D],
                        )
                        for t4 in range(4):
                            nc.vector.tensor_scalar(
                                stgs[t4][:, h * D:(h + 1) * D],
                                ap[:, t4 * (D + 1):t4 * (D + 1) + D],
                                r[:, t4:t4 + 1],
                                None,
                                Mult,
                            )
                        if h == 1:
                            for t4 in range(4):
                                nc.sync.dma_start(
                                    out_d[tcq * 512 + t4 * 128:
                                          tcq * 512 + (t4 + 1) * 128,
                                          p * 128:(p + 1) * 128],
                                    stgs[t4][:],
                                )

                    return run

                return [mk(c) for c in range(8)]

            def attn_unit(p, tcq, pt):
                for f in attn_emit(p, tcq, pt):
                    f()

            # Program order is semantic order under Tile (WAR/RAW follow it),
            # and it is also the scheduler's priority order. Software-pipeline
            # the softmax: emit scores(u+1) before attn(u) so ACT never
            # starves at a unit boundary; slot filler work (V projection,
            # pair-1 QK, deferred QT-0 chunks) right after the scores that
            # precede its first use.
            # Minimal critical path to the first exp: QT0[tc0], KT0[tc0],
            # then unit-0 scores. All remaining projection work (KT0 tails,
            # QT0 tails, V, pair-1 QK) is spread through the score s-loops
            # as hook filler so PE keeps ACT fed instead of lumping
            # projections between units. attn runs two units behind scores
            # (pt pool bufs >= 3). Everything is emitted before its first
            # program-order use (Tile semantics follow program order).
            proj_qk(0, wq, QT, [0])
            proj_qk(0, wk, KT, [0], eng="act")
            units = [(p, tcq) for p in range(2) for tcq in range(NTC)]
            qk0 = lambda w, d, tcs: (lambda: proj_qk(
                0, w, d, tcs, eng="act" if d is KT else None))
            qk1 = lambda w, d, tcs: (lambda: proj_qk(
                1, w, d, tcs, eng="act" if d is KT else None))
            pv = lambda ts: (lambda: proj_v(ts))
            if NTC == 4:
                # all V projections land in unit 1: the lag-2 attn chains
                # interleaved into unit 2 read every V s-tile, and Tile
                # program order is semantic order
                hooks = {
                    0: {4: [qk0(wk, KT, [1])], 8: [qk0(wk, KT, [2])],
                        12: [qk0(wk, KT, [3])]},
                    1: {0: [qk0(wq, QT, [1])], 2: [pv(range(0, 4))],
                        4: [pv(range(4, 8))], 6: [pv(range(8, 12))],
                        8: [pv(range(12, 16))], 12: [qk0(wq, QT, [2])]},
                    2: {0: [qk0(wq, QT, [3])], 10: [qk1(wk, KT, [0])],
                        14: [qk1(wk, KT, [1])]},
                    3: {0: [qk1(wk, KT, [2])], 3: [qk1(wk, KT, [3])],
                        6: [qk1(wq, QT, [0])], 10: [qk1(wq, QT, [1])],
                        14: [qk1(wq, QT, [2])]},
                    4: {4: [qk1(wq, QT, [3])]},
                }
                fillers = {}
            else:
                hooks = {0: {4 * c: [qk0(wk, KT, [c])] for c in range(1, NTC)}}
                fillers = {0: [qk0(wq, QT, range(1, NTC)),
                               pv(range(NT))]}
                fillers.setdefault(min(1, NTC - 1), []).extend((
                    qk1(wk, KT, range(NTC)),))
                fillers.setdefault(min(2, NTC - 1), []).append(
                    qk1(wq, QT, range(NTC)))
            pending = []  # [(p, tcq, pt)] up to two units behind
            for i, (p, tcq) in enumerate(units):
                hk = {st: list(fs) for st, fs in (hooks.get(i) or {}).items()}
                if len(pending) == 2:
                    args = pending.pop(0)
                    if not skip_attn:
                        # interleave the lag-2 unit's attn chains between
                        # score groups (odd s-tiles) so they sit in the PE
                        # queue inside the exp-paced scores loop and fill
                        # the bank-wait gaps
                        for c, f in enumerate(attn_emit(*args)):
                            hk.setdefault(min(2 * c + 1, NT - 1), []).append(f)
                pt = scores_unit(p, tcq, hk)
                for f in fillers.get(i, []):
                    f()
                pending.append((p, tcq, pt))
            for args in pending:
                if not skip_attn:
                    attn_unit(*args)

    nc.compile()
    return nc


def _shard_inputs(x, w_Q, w_K, w_V):
    bf = ml_dtypes.bfloat16
    in_maps = []
    for c in range(NCORES):
        b, g = divmod(c, NCORES // B)
        cols = slice(g * GC, (g + 1) * GC)
        in_maps.append({
            "xT": np.ascontiguousarray(np.asarray(x)[b].T).astype(bf),
            "wq": np.ascontiguousarray(np.asarray(w_Q)[:, cols]).astype(bf),
            "wk": np.ascontiguousarray(np.asarray(w_K)[:, cols]).astype(bf),
            "wv": np.ascontiguousarray(np.asarray(w_V)[:, cols]).astype(bf),
        })
    return in_maps


def kernel(x, w_Q, w_K, w_V, _trace=False, _tmpdir=None):
    from concourse.bass_utils import run_bass_kernel_spmd

    global _cached_nc
    if _cached_nc is None:
        _cached_nc = _build_program(T)
    in_maps = _shard_inputs(x, w_Q, w_K, w_V)
    res = run_bass_kernel_spmd(
        _cached_nc, in_maps, list(range(NCORES)),
        trace=_trace, tmpdir=_tmpdir,
    )
    out = np.empty((B, T, E), np.float32)
    for c in range(NCORES):
        b, g = divmod(c, NCORES // B)
        out[b, :, g * GC:(g + 1) * GC] = res.results[c]["out"]
    if _trace:
        return out, res
    return out



# revision 22
# speedup vs baseline: 1.1064x; 1.1064x over previous
"""Multi-head attention (B=2, T=2048, E=1024, H=16) on 8 TRN2 NeuronCores.

Sharding: core c handles batch c//4 and head group c%4 (4 heads of 64 dims
-> 256 columns of w_Q/w_K/w_V and of the output). Pure SPMD, no collectives:
every core runs the same NEFF on its own input shard.

Per-core kernel (all matmul operands bf16, PSUM/softmax math fp32):
  xT [E, T] (host pre-transposed), wq/wk/wv [E, 256]
  1. QT/KT per head-pair p: [128, T] = (w pair-slice)^T @ xT   (PE)
  2. V per s-tile: [128, 4*65] with a ones column per head     (PE + DVE copy)
  3. scores transposed per head: ST[s, t] = K Q^T, two heads packed into
     PE row groups (K=64 each) writing one [128, 1024] PSUM tile
  4. exp via ACT straight from PSUM, scale=1/8 folded into the activation
     affine, bf16 out -> PT
  5. attn: out[t,65] = PT_slice^T @ V_aug accumulated over 16 s-chunks;
     col 64 = softmax denominator (from the ones column)
  6. normalize: DVE reciprocal + per-partition tensor_scalar mul -> fp32 out
"""

import numpy as np
import ml_dtypes

B, T, E, H = 2, 2048, 1024, 16
D = 64          # head dim
HG = 4          # heads per core
GC = HG * D     # 256 output columns per core
NCORES = 8

_cached_nc = None


def _build_program(seq: int = T, reps: int = 1, skip_attn=False, skip_exp=False):
    """reps>1 emits the body multiple times in one NEFF (timing only).
    skip_attn/skip_exp build ablation variants for HW phase attribution."""
    import concourse.bacc as bacc
    import concourse.tile as tile
    from concourse import mybir

    bf16 = mybir.dt.bfloat16
    f32 = mybir.dt.float32
    i16 = mybir.dt.int16
    Exp = mybir.ActivationFunctionType.Exp
    Mult = mybir.AluOpType.mult
    Add = mybir.AluOpType.add
    # int16 Schraudolph fast-exp: bits16 = 2^7*(x*log2e*0.125 + 127 - c),
    # written as int16 and bitcast to bf16 (bf16 shares fp32's exponent
    # layout). One tensor_scalar per group, no convert-copy needed, so DVE
    # and GPSIMD form two independent exp lanes next to ACT's.
    S16_C1 = (1 << 7) * 1.4426950408889634 * 0.125
    S16_C2 = (1 << 7) * (127.0 - 0.04367744)
    EXP_ACT_GROUPS = 9   # 1024-wide groups [0,9) -> ACT
    # groups [9,16) -> DVE int16 lane (GPSIMD cannot read PSUM)

    NT = seq // 128     # s-tiles / t-tiles
    NTC = seq // 512    # 512-wide t-chunks
    KO = E // 128       # contraction chunks for projections

    nc = bacc.Bacc(
        "TRN2", target_bir_lowering=False, debug=False, num_devices=NCORES
    )

    xT_d = nc.dram_tensor("xT", [E, seq], bf16, kind="ExternalInput")
    wq_d = nc.dram_tensor("wq", [E, GC], bf16, kind="ExternalInput")
    wk_d = nc.dram_tensor("wk", [E, GC], bf16, kind="ExternalInput")
    wv_d = nc.dram_tensor("wv", [E, GC], bf16, kind="ExternalInput")
    out_d = nc.dram_tensor("out", [seq, GC], f32, kind="ExternalOutput")

    with tile.TileContext(nc) as tc:
        with (
            tc.tile_pool(name="singles", bufs=1) as singles,
            tc.tile_pool(name="pt", bufs=3) as ptp,
            tc.tile_pool(name="stage", bufs=8) as stagep,
            tc.tile_pool(name="recip", bufs=8) as recipp,
            # PSUM budget (8 banks): scores 2x[128,1024] (4) + attn 2 + proj 2
            # (proj/attn double-buffered so PSUM->SBUF drains overlap the
            # next accumulation chain instead of stalling PE)
            tc.tile_pool(name="proj_ps", bufs=2, space="PSUM") as proj_ps,
            tc.tile_pool(name="score_ps", bufs=2, space="PSUM") as score_ps,
            tc.tile_pool(name="attn_ps", bufs=2, space="PSUM") as attn_ps,
        ):
          for _rep in range(reps):
            # ---- load inputs (weights first: first matmul needs wk+xT[0]) --
            wq = singles.tile([128, KO, GC], bf16)
            wk = singles.tile([128, KO, GC], bf16)
            wv = singles.tile([128, KO, GC], bf16)
            nc.sync.dma_start(wk[:], wk_d[:].rearrange("(ko p) c -> p ko c", p=128))
            nc.sync.dma_start(wq[:], wq_d[:].rearrange("(ko p) c -> p ko c", p=128))
            # xT streamed t-chunk-major so the first projection's k-loop only
            # waits for the first 1MB, not the full 4MB.
            xT = singles.tile([128, KO, seq], bf16)
            for tcq in range(NTC):
                for k in range(KO):
                    nc.sync.dma_start(
                        xT[:, k, tcq * 512:(tcq + 1) * 512],
                        xT_d[k * 128:(k + 1) * 128, tcq * 512:(tcq + 1) * 512],
                    )
            nc.sync.dma_start(wv[:], wv_d[:].rearrange("(ko p) c -> p ko c", p=128))

            # QT/KT: [128, pair, seq]; partitions 0-63 head 2p, 64-127 head 2p+1
            QT = singles.tile([128, 2, seq], bf16)
            KT = singles.tile([128, 2, seq], bf16)
            # V with ones col per head: [128, s-tile, 4*65]
            V = singles.tile([128, NT, HG * (D + 1)], bf16)
            # only the per-head ones-columns need the memset; the D data
            # columns are fully overwritten by proj_v
            nc.vector.memset(
                V[:].rearrange("p s (h c) -> p s h c", h=HG)[:, :, :, D:], 1.0
            )

            def proj_qk(p, w_sb, dst, tcs, eng=None):
                """Project t-chunks `tcs` of QT or KT for head-pair p.
                eng picks the PSUM->SBUF copy engine (DVE default; ACT for
                KT to balance the two engines' load)."""
                for tcq in tcs:
                    ps = proj_ps.tile([128, 512], f32, tag="proj")
                    for k in range(KO):
                        nc.tensor.matmul(
                            ps[:],
                            lhsT=w_sb[:, k, p * 128:(p + 1) * 128],
                            rhs=xT[:, k, tcq * 512:(tcq + 1) * 512],
                            start=(k == 0),
                            stop=(k == KO - 1),
                        )
                    dst_ap = dst[:, p, tcq * 512:(tcq + 1) * 512]
                    if eng == "act":
                        nc.scalar.copy(out=dst_ap, in_=ps[:])
                    else:
                        nc.vector.tensor_copy(out=dst_ap, in_=ps[:])

            def proj_v(tiles):
                for st in tiles:
                    ps = proj_ps.tile([128, 512], f32, tag="proj")
                    for k in range(KO):
                        nc.tensor.matmul(
                            ps[:, :GC],
                            lhsT=xT[:, k, st * 128:(st + 1) * 128],
                            rhs=wv[:, k, :],
                            start=(k == 0),
                            stop=(k == KO - 1),
                        )
                    nc.scalar.copy(
                        out=V[:, st].rearrange("p (h c) -> p h c", h=HG)[:, :, :D],
                        in_=ps[:, :GC].rearrange("p (h c) -> p h c", h=HG),
                    )

            def scores_unit(p, tcq, hooks=None):
                """ST = K Q^T (both heads row-packed) + exp -> PT tile.

                PT layout is flat [128, NT*1024]: 512-wide bank-write j=2*st+h
                lands at elem offset j*512 (= st*1024 + h*512). Exps are
                grouped 2 banks at a time (FD=1024). hooks[st] emits filler
                work just before score s-tile st."""
                pt = ptp.tile([128, NT * 1024], bf16, tag="pt")
                sc = None
                for st in range(NT):
                    for f in (hooks or {}).get(st, []):
                        f()
                    for h in range(2):
                        j = 2 * st + h
                        if j % 2 == 0:
                            gj = j
                            sc = score_ps.tile([128, 1024], f32, tag="score")
                        nc.tensor.matmul(
                            sc[:, (j - gj) * 512:(j - gj + 1) * 512],
                            lhsT=KT[h * 64:(h + 1) * 64, p,
                                    st * 128:(st + 1) * 128],
                            rhs=QT[h * 64:(h + 1) * 64, p,
                                   tcq * 512:(tcq + 1) * 512],
                            start=True,
                            stop=True,
                        )
                        if skip_exp:
                            continue
                        if j == gj + 1:
                            dst = pt[:, gj * 512:(gj + 2) * 512]
                            # Two exp lanes; early groups on ACT (attn
                            # consumes PT in s-order and ACT's groups finish
                            # serially first while DVE fills the tail).
                            if gj // 2 < EXP_ACT_GROUPS:
                                nc.scalar.activation(
                                    out=dst, in_=sc[:],
                                    func=Exp, scale=0.125,
                                )
                            else:
                                nc.vector.tensor_scalar(
                                    dst.bitcast(i16), sc[:],
                                    S16_C1, S16_C2, Mult, Add,
                                )
                return pt

            def attn_emit(p, tcq, pt):
                """attn = PT^T @ V_aug accumulated over s, then normalize.
                Returns 8 closures, one per (h, tt) accumulation chain (heads
                sequential, one open accumulation group per PSUM bank), so
                chains can be interleaved between score groups as PE filler.
                Normalize + output DMA ride on the tt=3 chains."""
                stgs, aps = [], {}

                def mk(c):
                    h, tt = divmod(c, 4)
                    hh = p * 2 + h

                    def run():
                        if c == 0:
                            stgs.extend(
                                stagep.tile([128, 128], f32, tag="stage",
                                            name=f"stg{t}")
                                for t in range(4)
                            )
                        if tt == 0:
                            aps[h] = attn_ps.tile([128, 4 * (D + 1)], f32,
                                                  tag="attn", name=f"ap{h}")
                        ap = aps[h]
                        for st in range(NT):
                            nc.tensor.matmul(
                                ap[:, tt * (D + 1):(tt + 1) * (D + 1)],
                                lhsT=pt[:, st * 1024 + h * 512 + tt * 128:
                                        st * 1024 + h * 512 + (tt + 1) * 128],
                                rhs=V[:, st, hh * (D + 1):(hh + 1) * (D + 1)],
                                start=(st == 0),
                                stop=(st == NT - 1),
                            )
                        if tt != 3:
                            return
                        # one strided reciprocal over the 4 denom columns
                        r = recipp.tile([128, 4], f32, tag="recip")
                        nc.vector.reciprocal(
                            out=r[:],
                            in_=ap[:].rearrange(
                                "p (t c) -> p t c", c=D + 1)[:, :, D],
                        )
                        for t4 in range(4):
                            nc.vector.tensor_scalar(
                                stgs[t4][:, h * D:(h + 1) * D],
                                ap[:, t4 * (D + 1):t4 * (D + 1) + D],
                                r[:, t4:t4 + 1],
                                None,
                                Mult,
                            )
                        if h == 1:
                            for t4 in range(4):
                                nc.sync.dma_start(
                                    out_d[tcq * 512 + t4 * 128:
                                          tcq * 512 + (t4 + 1) * 128,
                                          p * 128:(p + 1) * 128],
                                    stgs[t4][:],
                                )

                    return run

                return [mk(c) for c in range(8)]

            def attn_unit(p, tcq, pt):
                for f in attn_emit(p, tcq, pt):
                    f()

            # Program order is semantic order under Tile (WAR/RAW follow it),
            # and it is also the scheduler's priority order. Software-pipeline
            # the softmax: emit scores(u+1) before attn(u) so ACT never
            # starves at a unit boundary; slot filler work (V projection,
            # pair-1 QK, deferred QT-0 chunks) right after the scores that
            # precede its first use.
            # Minimal critical path to the first exp: QT0[tc0], KT0[tc0],
            # then unit-0 scores. All remaining projection work (KT0 tails,
            # QT0 tails, V, pair-1 QK) is spread through the score s-loops
            # as hook filler so PE keeps ACT fed instead of lumping
            # projections between units. attn runs two units behind scores
            # (pt pool bufs >= 3). Everything is emitted before its first
            # program-order use (Tile semantics follow program order).
            proj_qk(0, wq, QT, [0])
            proj_qk(0, wk, KT, [0], eng="act")
            units = [(p, tcq) for p in range(2) for tcq in range(NTC)]
            qk0 = lambda w, d, tcs: (lambda: proj_qk(
                0, w, d, tcs, eng="act" if d is KT else None))
            qk1 = lambda w, d, tcs: (lambda: proj_qk(
                1, w, d, tcs, eng="act" if d is KT else None))
            pv = lambda ts: (lambda: proj_v(ts))
            if NTC == 4:
                # all V projections land in unit 1: the lag-2 attn chains
                # interleaved into unit 2 read every V s-tile, and Tile
                # program order is semantic order
                hooks = {
                    0: {4: [qk0(wk, KT, [1])], 8: [qk0(wk, KT, [2])],
                        12: [qk0(wk, KT, [3])]},
                    1: {0: [qk0(wq, QT, [1])], 2: [pv(range(0, 4))],
                        4: [pv(range(4, 8))], 6: [pv(range(8, 12))],
                        8: [pv(range(12, 16))], 12: [qk0(wq, QT, [2])]},
                    2: {0: [qk0(wq, QT, [3])], 10: [qk1(wk, KT, [0])],
                        14: [qk1(wk, KT, [1])]},
                    3: {0: [qk1(wk, KT, [2])], 3: [qk1(wk, KT, [3])],
                        6: [qk1(wq, QT, [0])], 10: [qk1(wq, QT, [1])],
                        14: [qk1(wq, QT, [2])]},
                    4: {4: [qk1(wq, QT, [3])]},
                }
                fillers = {}
            else:
                hooks = {0: {4 * c: [qk0(wk, KT, [c])] for c in range(1, NTC)}}
                fillers = {0: [qk0(wq, QT, range(1, NTC)),
                               pv(range(NT))]}
                fillers.setdefault(min(1, NTC - 1), []).extend((
                    qk1(wk, KT, range(NTC)),))
                fillers.setdefault(min(2, NTC - 1), []).append(
                    qk1(wq, QT, range(NTC)))
            pending = []  # [(p, tcq, pt)] up to two units behind
            for i, (p, tcq) in enumerate(units):
                hk = {st: list(fs) for st, fs in (hooks.get(i) or {}).items()}
                if len(pending) == 2:
                    args = pending.pop(0)
                    if not skip_attn:
                        # interleave the lag-2 unit's attn chains between
                        # score groups (odd s-tiles) so they sit in the PE
                        # queue inside the exp-paced scores loop and fill
                        # the bank-wait gaps
                        for c, f in enumerate(attn_emit(*args)):
                            hk.setdefault(min(2 * c + 1, NT - 1), []).append(f)
                pt = scores_unit(p, tcq, hk)
                for f in fillers.get(i, []):
                    f()
                pending.append((p, tcq, pt))
            for args in pending:
                if not skip_attn:
                    attn_unit(*args)

    nc.compile()
    return nc


def _shard_inputs(x, w_Q, w_K, w_V):
    bf = ml_dtypes.bfloat16
    in_maps = []
    for c in range(NCORES):
        b, g = divmod(c, NCORES // B)
        cols = slice(g * GC, (g + 1) * GC)
        in_maps.append({
            "xT": np.ascontiguousarray(np.asarray(x)[b].T).astype(bf),
            "wq": np.ascontiguousarray(np.asarray(w_Q)[:, cols]).astype(bf),
            "wk": np.ascontiguousarray(np.asarray(w_K)[:, cols]).astype(bf),
            "wv": np.ascontiguousarray(np.asarray(w_V)[:, cols]).astype(bf),
        })
    return in_maps


def kernel(x, w_Q, w_K, w_V, _trace=False, _tmpdir=None):
    from concourse.bass_utils import run_bass_kernel_spmd

    global _cached_nc
    if _cached_nc is None:
        _cached_nc = _build_program(T)
    in_maps = _shard_inputs(x, w_Q, w_K, w_V)
    res = run_bass_kernel_spmd(
        _cached_nc, in_maps, list(range(NCORES)),
        trace=_trace, tmpdir=_tmpdir,
    )
    out = np.empty((B, T, E), np.float32)
    for c in range(NCORES):
        b, g = divmod(c, NCORES // B)
        out[b, :, g * GC:(g + 1) * GC] = res.results[c]["out"]
    if _trace:
        return out, res
    return out



# revision 25
# speedup vs baseline: 1.2884x; 1.1645x over previous
"""Multi-head attention (B=2, T=2048, E=1024, H=16) on 8 TRN2 NeuronCores.

Sharding: core c handles batch c//4 and head group c%4 (4 heads of 64 dims
-> 256 columns of w_Q/w_K/w_V and of the output). Pure SPMD, no collectives:
every core runs the same NEFF on its own input shard.

Per-core kernel (all matmul operands bf16, PSUM/softmax math fp32):
  xT [E, T] (host pre-transposed), wq/wk/wv [E, 256]
  1. QT/KT per head-pair p: [128, T] = (w pair-slice)^T @ xT   (PE;
     KT/V PSUM->SBUF drains on ACT, QT on DVE, to balance the two)
  2. V per s-tile: [128, 4*65] with a ones column per head
  3. scores transposed per head: ST[s, t] = K Q^T, two heads packed into
     PE row groups (K=64 each) writing one [128, 1024] PSUM tile
  4. exp in two concurrent lanes per 1024-wide group: early groups on ACT
     (Exp activation, scale=1/8 folded in, bf16 out), tail groups on DVE
     via an int16 Schraudolph 2^x bit trick (bits16 = 2^7*(s*log2e/8 +
     127 - c) written as int16, bitcast bf16) -> PT
  5. attn: out[t,65] = PT_slice^T @ V_aug accumulated over 16 s-chunks;
     col 64 = softmax denominator (from the ones column). The 8 (h,tt)
     accumulation chains of the lag-2 unit are interleaved between score
     groups as PE-queue filler; proj/attn PSUM pools are double-buffered
     so drains overlap the next chain.
  6. normalize: DVE strided reciprocal + per-partition tensor_scalar -> f32
"""

import numpy as np
import ml_dtypes

B, T, E, H = 2, 2048, 1024, 16
D = 64          # head dim
HG = 4          # heads per core
GC = HG * D     # 256 output columns per core
NCORES = 8

_cached_nc = None


def _build_program(seq: int = T, reps: int = 1, skip_attn=False, skip_exp=False):
    """reps>1 emits the body multiple times in one NEFF (timing only).
    skip_attn/skip_exp build ablation variants for HW phase attribution."""
    import concourse.bacc as bacc
    import concourse.tile as tile
    from concourse import mybir

    bf16 = mybir.dt.bfloat16
    f32 = mybir.dt.float32
    i16 = mybir.dt.int16
    Exp = mybir.ActivationFunctionType.Exp
    Mult = mybir.AluOpType.mult
    Add = mybir.AluOpType.add
    # int16 Schraudolph fast-exp: bits16 = 2^7*(x*log2e*0.125 + 127 - c),
    # written as int16 and bitcast to bf16 (bf16 shares fp32's exponent
    # layout). One tensor_scalar per group, no convert-copy needed, so DVE
    # and GPSIMD form two independent exp lanes next to ACT's.
    S16_C1 = (1 << 7) * 1.4426950408889634 * 0.125
    S16_C2 = (1 << 7) * (127.0 - 0.04367744)
    # 1024-wide exp groups alternate ACT / DVE-int16 (9:7 per unit) so the
    # two lanes drain ADJACENT score banks concurrently — a blocked
    # assignment serializes them behind the 2-deep score tile pool.
    # (GPSIMD cannot read PSUM, so only these two lanes exist.)
    EXP_ACT_SET = frozenset({0, 2, 4, 6, 8, 10, 12, 14, 15})

    NT = seq // 128     # s-tiles / t-tiles
    NTC = seq // 512    # 512-wide t-chunks
    KO = E // 128       # contraction chunks for projections

    nc = bacc.Bacc(
        "TRN2", target_bir_lowering=False, debug=False, num_devices=NCORES
    )

    xT_d = nc.dram_tensor("xT", [E, seq], bf16, kind="ExternalInput")
    wq_d = nc.dram_tensor("wq", [E, GC], bf16, kind="ExternalInput")
    wk_d = nc.dram_tensor("wk", [E, GC], bf16, kind="ExternalInput")
    wv_d = nc.dram_tensor("wv", [E, GC], bf16, kind="ExternalInput")
    out_d = nc.dram_tensor("out", [seq, GC], f32, kind="ExternalOutput")

    with tile.TileContext(nc) as tc:
        with (
            tc.tile_pool(name="singles", bufs=1) as singles,
            tc.tile_pool(name="pt", bufs=3) as ptp,
            tc.tile_pool(name="stage", bufs=8) as stagep,
            tc.tile_pool(name="recip", bufs=8) as recipp,
            # PSUM budget (8 banks): scores 2x[128,1024] (4) + attn 2 + proj 2
            # (proj/attn double-buffered so PSUM->SBUF drains overlap the
            # next accumulation chain instead of stalling PE)
            tc.tile_pool(name="proj_ps", bufs=2, space="PSUM") as proj_ps,
            tc.tile_pool(name="score_ps", bufs=2, space="PSUM") as score_ps,
            tc.tile_pool(name="attn_ps", bufs=2, space="PSUM") as attn_ps,
        ):
          for _rep in range(reps):
            # ---- load inputs (weights first: first matmul needs wk+xT[0]) --
            wq = singles.tile([128, KO, GC], bf16)
            wk = singles.tile([128, KO, GC], bf16)
            wv = singles.tile([128, KO, GC], bf16)
            nc.sync.dma_start(wk[:], wk_d[:].rearrange("(ko p) c -> p ko c", p=128))
            nc.sync.dma_start(wq[:], wq_d[:].rearrange("(ko p) c -> p ko c", p=128))
            # xT streamed t-chunk-major so the first projection's k-loop only
            # waits for the first 1MB, not the full 4MB.
            xT = singles.tile([128, KO, seq], bf16)
            for tcq in range(NTC):
                for k in range(KO):
                    nc.sync.dma_start(
                        xT[:, k, tcq * 512:(tcq + 1) * 512],
                        xT_d[k * 128:(k + 1) * 128, tcq * 512:(tcq + 1) * 512],
                    )
            nc.sync.dma_start(wv[:], wv_d[:].rearrange("(ko p) c -> p ko c", p=128))

            # QT/KT: [128, pair, seq]; partitions 0-63 head 2p, 64-127 head 2p+1
            QT = singles.tile([128, 2, seq], bf16)
            KT = singles.tile([128, 2, seq], bf16)
            # V with ones col per head: [128, s-tile, 4*65]
            V = singles.tile([128, NT, HG * (D + 1)], bf16)
            # only the per-head ones-columns need the memset; the D data
            # columns are fully overwritten by proj_v
            nc.vector.memset(
                V[:].rearrange("p s (h c) -> p s h c", h=HG)[:, :, :, D:], 1.0
            )

            def proj_qk(p, w_sb, dst, tcs, eng=None):
                """Project t-chunks `tcs` of QT or KT for head-pair p.
                eng picks the PSUM->SBUF copy engine (DVE default; ACT for
                KT to balance the two engines' load)."""
                for tcq in tcs:
                    ps = proj_ps.tile([128, 512], f32, tag="proj")
                    for k in range(KO):
                        nc.tensor.matmul(
                            ps[:],
                            lhsT=w_sb[:, k, p * 128:(p + 1) * 128],
                            rhs=xT[:, k, tcq * 512:(tcq + 1) * 512],
                            start=(k == 0),
                            stop=(k == KO - 1),
                        )
                    dst_ap = dst[:, p, tcq * 512:(tcq + 1) * 512]
                    if eng == "act":
                        nc.scalar.copy(out=dst_ap, in_=ps[:])
                    else:
                        nc.vector.tensor_copy(out=dst_ap, in_=ps[:])

            def proj_v(tiles):
                for st in tiles:
                    ps = proj_ps.tile([128, 512], f32, tag="proj")
                    for k in range(KO):
                        nc.tensor.matmul(
                            ps[:, :GC],
                            lhsT=xT[:, k, st * 128:(st + 1) * 128],
                            rhs=wv[:, k, :],
                            start=(k == 0),
                            stop=(k == KO - 1),
                        )
                    nc.scalar.copy(
                        out=V[:, st].rearrange("p (h c) -> p h c", h=HG)[:, :, :D],
                        in_=ps[:, :GC].rearrange("p (h c) -> p h c", h=HG),
                    )

            def scores_unit(p, tcq, hooks=None):
                """ST = K Q^T (both heads row-packed) + exp -> PT tile.

                PT layout is flat [128, NT*1024]: 512-wide bank-write j=2*st+h
                lands at elem offset j*512 (= st*1024 + h*512). Exps are
                grouped 2 banks at a time (FD=1024). hooks[st] emits filler
                work just before score s-tile st."""
                pt = ptp.tile([128, NT * 1024], bf16, tag="pt")
                sc = None
                for st in range(NT):
                    for f in (hooks or {}).get(st, []):
                        f()
                    for h in range(2):
                        j = 2 * st + h
                        if j % 2 == 0:
                            gj = j
                            sc = score_ps.tile([128, 1024], f32, tag="score")
                        nc.tensor.matmul(
                            sc[:, (j - gj) * 512:(j - gj + 1) * 512],
                            lhsT=KT[h * 64:(h + 1) * 64, p,
                                    st * 128:(st + 1) * 128],
                            rhs=QT[h * 64:(h + 1) * 64, p,
                                   tcq * 512:(tcq + 1) * 512],
                            start=True,
                            stop=True,
                        )
                        if skip_exp:
                            continue
                        if j == gj + 1:
                            dst = pt[:, gj * 512:(gj + 2) * 512]
                            # Two exp lanes, alternating per group.
                            if (gj // 2) % 16 in EXP_ACT_SET:
                                nc.scalar.activation(
                                    out=dst, in_=sc[:],
                                    func=Exp, scale=0.125,
                                )
                            else:
                                nc.vector.tensor_scalar(
                                    dst.bitcast(i16), sc[:],
                                    S16_C1, S16_C2, Mult, Add,
                                )
                return pt

            def attn_emit(p, tcq, pt):
                """attn = PT^T @ V_aug accumulated over s, then normalize.
                Returns 8 closures, one per (h, tt) accumulation chain (heads
                sequential, one open accumulation group per PSUM bank), so
                chains can be interleaved between score groups as PE filler.
                Normalize + output DMA ride on the tt=3 chains."""
                stgs, aps = [], {}

                def mk(c):
                    h, tt = divmod(c, 4)
                    hh = p * 2 + h

                    def run():
                        if c == 0:
                            stgs.extend(
                                stagep.tile([128, 128], f32, tag="stage",
                                            name=f"stg{t}")
                                for t in range(4)
                            )
                        if tt == 0:
                            aps[h] = attn_ps.tile([128, 4 * (D + 1)], f32,
                                                  tag="attn", name=f"ap{h}")
                        ap = aps[h]
                        for st in range(NT):
                            nc.tensor.matmul(
                                ap[:, tt * (D + 1):(tt + 1) * (D + 1)],
                                lhsT=pt[:, st * 1024 + h * 512 + tt * 128:
                                        st * 1024 + h * 512 + (tt + 1) * 128],
                                rhs=V[:, st, hh * (D + 1):(hh + 1) * (D + 1)],
                                start=(st == 0),
                                stop=(st == NT - 1),
                            )
                        if tt != 3:
                            return
                        # one strided reciprocal over the 4 denom columns
                        r = recipp.tile([128, 4], f32, tag="recip")
                        nc.vector.reciprocal(
                            out=r[:],
                            in_=ap[:].rearrange(
                                "p (t c) -> p t c", c=D + 1)[:, :, D],
                        )
                        for t4 in range(4):
                            nc.vector.tensor_scalar(
                                stgs[t4][:, h * D:(h + 1) * D],
                                ap[:, t4 * (D + 1):t4 * (D + 1) + D],
                                r[:, t4:t4 + 1],
                                None,
                                Mult,
                            )
                        if h == 1:
                            for t4 in range(4):
                                nc.sync.dma_start(
                                    out_d[tcq * 512 + t4 * 128:
                                          tcq * 512 + (t4 + 1) * 128,
                                          p * 128:(p + 1) * 128],
                                    stgs[t4][:],
                                )

                    return run

                return [mk(c) for c in range(8)]

            def attn_unit(p, tcq, pt):
                for f in attn_emit(p, tcq, pt):
                    f()

            # Program order is semantic order under Tile (WAR/RAW follow it),
            # and it is also the scheduler's priority order. Software-pipeline
            # the softmax: emit scores(u+1) before attn(u) so ACT never
            # starves at a unit boundary; slot filler work (V projection,
            # pair-1 QK, deferred QT-0 chunks) right after the scores that
            # precede its first use.
            # Minimal critical path to the first exp: QT0[tc0], KT0[tc0],
            # then unit-0 scores. All remaining projection work (KT0 tails,
            # QT0 tails, V, pair-1 QK) is spread through the score s-loops
            # as hook filler so PE keeps ACT fed instead of lumping
            # projections between units. attn runs two units behind scores
            # (pt pool bufs >= 3). Everything is emitted before its first
            # program-order use (Tile semantics follow program order).
            proj_qk(0, wq, QT, [0])
            proj_qk(0, wk, KT, [0], eng="act")
            units = [(p, tcq) for p in range(2) for tcq in range(NTC)]
            qk0 = lambda w, d, tcs: (lambda: proj_qk(
                0, w, d, tcs, eng="act" if d is KT else None))
            qk1 = lambda w, d, tcs: (lambda: proj_qk(
                1, w, d, tcs, eng="act" if d is KT else None))
            pv = lambda ts: (lambda: proj_v(ts))
            if NTC == 4:
                # all V projections land in unit 1: the lag-2 attn chains
                # interleaved into unit 2 read every V s-tile, and Tile
                # program order is semantic order
                hooks = {
                    0: {4: [qk0(wk, KT, [1])], 8: [qk0(wk, KT, [2])],
                        12: [qk0(wk, KT, [3])]},
                    1: {0: [qk0(wq, QT, [1])], 2: [pv(range(0, 4))],
                        4: [pv(range(4, 8))], 6: [pv(range(8, 12))],
                        8: [pv(range(12, 16))], 12: [qk0(wq, QT, [2])]},
                    2: {0: [qk0(wq, QT, [3])], 10: [qk1(wk, KT, [0])],
                        14: [qk1(wk, KT, [1])]},
                    3: {0: [qk1(wk, KT, [2])], 3: [qk1(wk, KT, [3])],
                        6: [qk1(wq, QT, [0])], 10: [qk1(wq, QT, [1])],
                        14: [qk1(wq, QT, [2])]},
                    4: {4: [qk1(wq, QT, [3])]},
                }
                fillers = {}
            else:
                hooks = {0: {4 * c: [qk0(wk, KT, [c])] for c in range(1, NTC)}}
                fillers = {0: [qk0(wq, QT, range(1, NTC)),
                               pv(range(NT))]}
                fillers.setdefault(min(1, NTC - 1), []).extend((
                    qk1(wk, KT, range(NTC)),))
                fillers.setdefault(min(2, NTC - 1), []).append(
                    qk1(wq, QT, range(NTC)))
            pending = []  # [(p, tcq, pt)] up to two units behind
            for i, (p, tcq) in enumerate(units):
                hk = {st: list(fs) for st, fs in (hooks.get(i) or {}).items()}
                if len(pending) == 2:
                    args = pending.pop(0)
                    if not skip_attn:
                        # interleave the lag-2 unit's attn chains between
                        # score groups (odd s-tiles) so they sit in the PE
                        # queue inside the exp-paced scores loop and fill
                        # the bank-wait gaps
                        for c, f in enumerate(attn_emit(*args)):
                            hk.setdefault(min(2 * c + 1, NT - 1), []).append(f)
                pt = scores_unit(p, tcq, hk)
                for f in fillers.get(i, []):
                    f()
                pending.append((p, tcq, pt))
            for args in pending:
                if not skip_attn:
                    attn_unit(*args)

    nc.compile()
    return nc


def _shard_inputs(x, w_Q, w_K, w_V):
    bf = ml_dtypes.bfloat16
    in_maps = []
    for c in range(NCORES):
        b, g = divmod(c, NCORES // B)
        cols = slice(g * GC, (g + 1) * GC)
        in_maps.append({
            "xT": np.ascontiguousarray(np.asarray(x)[b].T).astype(bf),
            "wq": np.ascontiguousarray(np.asarray(w_Q)[:, cols]).astype(bf),
            "wk": np.ascontiguousarray(np.asarray(w_K)[:, cols]).astype(bf),
            "wv": np.ascontiguousarray(np.asarray(w_V)[:, cols]).astype(bf),
        })
    return in_maps


def kernel(x, w_Q, w_K, w_V, _trace=False, _tmpdir=None):
    from concourse.bass_utils import run_bass_kernel_spmd

    global _cached_nc
    if _cached_nc is None:
        _cached_nc = _build_program(T)
    in_maps = _shard_inputs(x, w_Q, w_K, w_V)
    res = run_bass_kernel_spmd(
        _cached_nc, in_maps, list(range(NCORES)),
        trace=_trace, tmpdir=_tmpdir,
    )
    out = np.empty((B, T, E), np.float32)
    for c in range(NCORES):
        b, g = divmod(c, NCORES // B)
        out[b, :, g * GC:(g + 1) * GC] = res.results[c]["out"]
    if _trace:
        return out, res
    return out

